# revision 69
# baseline (speedup 1.0000x reference)
"""Trainium2 Bass kernel for single-query cross-attention pooling
(segment softmax over equal-sized node segments, 8-way node/graph sharded).

Self-contained: hardcodes N=262144, D=256, H=8, G=1024, 8 cores.

Algorithm (per core, 128 graphs / 32768 nodes, all device math in fp16 with
fp32 PSUM accumulation):
  scores[n,h] = x[n,:] @ A2[:,h]        A2 = scale * Wk_h^T q_h  (host-folded;
                                        per-head additive consts cancel in the
                                        segment softmax, so they are dropped)
  p = exp(scores - U)                   U = uniform host bound, cancels too
  Z[g,h,:] = sum_{n in g} p[n,h]*[x[n,:] | 1]   (pool BEFORE projecting:
                                        projection commutes with the pooled sum)
  out[g,:] = sum_h (Z[g,h,:256]/Z[g,h,256]) @ CH_h + bout
                                        CH_h = Wv_h^T Wout_h^T  (host-folded)

The transposed copy of x (xT) is prepared on the host so scores can be computed
with x-tiles as the matmul stationary operand -> node-major scores, and no
on-device transposes of x are ever needed.  Total HBM traffic per core is
2 * 16 MB fp16 = the fp32-read-once roofline.
"""

import numpy as np

N_TOTAL = 262144
D = 256
H = 8
DH = 32
G_TOTAL = 1024
N_CORES = 8

N_LOC = N_TOTAL // N_CORES          # 32768 nodes per core
G_LOC = G_TOTAL // N_CORES          # 128 graphs per core
SEG = N_TOTAL // G_TOTAL            # 256 nodes per graph (equal segments)
NT = N_LOC // 128                   # 256 node tiles per core

_CACHE = {}


def _build_program(U, repeats=1, loop_iters=0, skip=(), tiny_out=False,
                   xnch=8, xtch=16, xt_f8=True, qcfg=("gpsimd", "sync",
                                                      "scalar")):
    """skip: subset of {'scores','z','xt_dma','xn_dma'} - diagnostic variants
    that elide parts of the per-tile work (output garbage; timing only).
    tiny_out: shrink the output tensor to [1,4] so relay transport per call
    is negligible (timing variants)."""
    import concourse.bass as bass
    import concourse.tile as tile
    from concourse import bacc, mybir
    from contextlib import ExitStack

    f16 = mybir.dt.float16
    f32 = mybir.dt.float32
    f8 = mybir.dt.float8e3 if xt_f8 else mybir.dt.float16
    f8e4 = mybir.dt.float8e4
    DR = mybir.MatmulPerfMode.DoubleRow
    import os
    xn_bufs = int(os.environ.get("XN_BUFS", "8"))
    xt_bufs = int(os.environ.get("XT_BUFS", "6"))

    nc = bacc.Bacc("TRN2", target_bir_lowering=False, debug=False,
                   num_devices=N_CORES)

    # x_nat is host-pre-tiled: [chunk, partition, tile_in_chunk * (D+1)] so a
    # chunk DMA is 128 contiguous 8224-byte descriptors instead of 2048
    # small ones (HWDGE descriptor-generation was the SP-seq bottleneck).
    XNCH = xnch          # x_nat tiles per DMA chunk
    XTCH = xtch          # xT tiles per DMA chunk
    x_nat = nc.dram_tensor("x_nat", [NT // XNCH, 128, XNCH * (D + 1)], f8,
                           kind="ExternalInput")
    # x_t / a2 are fp8-e3m4, pre-scaled x2 / x64 on the host; the combined
    # x128 on the scores is undone by the exp activation's scale=1/128.
    x_t = nc.dram_tensor("x_t", [D, N_LOC], f8, kind="ExternalInput")
    a2_d = nc.dram_tensor("a2", [D, 128], f8, kind="ExternalInput")
    ch_d = nc.dram_tensor("ch", [128, 32 * 128], f16, kind="ExternalInput")
    bout_d = nc.dram_tensor("bout", [128, 2], f32, kind="ExternalInput")
    ebias_d = nc.dram_tensor("ebias", [128, 2], f32, kind="ExternalInput")
    id16_d = nc.dram_tensor("id16", [128, 128], f16, kind="ExternalInput")
    # output is written TRANSPOSED [D, G_LOC] straight from the projection
    # psum (saves the final PE transposes); the host un-transposes.
    out_shape = [1, 4] if tiny_out else [D, G_LOC]
    out_d = nc.dram_tensor("out", out_shape, f32, kind="ExternalOutput")

    Exp = mybir.ActivationFunctionType.Exp
    Copy = mybir.ActivationFunctionType.Copy
    Ident = mybir.ActivationFunctionType.Identity

    with tile.TileContext(nc) as tc, ExitStack() as ctx:
        consts = ctx.enter_context(tc.tile_pool(name="consts", bufs=1))
        xn_pool = ctx.enter_context(tc.tile_pool(name="xn_pool", bufs=xn_bufs))
        xt_pool = ctx.enter_context(tc.tile_pool(name="xt_pool", bufs=xt_bufs))
        zn_pool = ctx.enter_context(tc.tile_pool(name="zn_pool", bufs=2))
        p_pool = ctx.enter_context(tc.tile_pool(name="p_pool", bufs=8))
        small = ctx.enter_context(tc.tile_pool(name="small", bufs=8))
        tailsb = ctx.enter_context(tc.tile_pool(name="tailsb", bufs=2))
        # PSUM: 8 banks total; tags share slots across main loop and tail
        bigps = ctx.enter_context(
            tc.tile_pool(name="bigps", bufs=2, space="PSUM"))
        smallps = ctx.enter_context(
            tc.tile_pool(name="smallps", bufs=2, space="PSUM"))
        zpsum = ctx.enter_context(
            tc.tile_pool(name="zpsum", bufs=2, space="PSUM"))
        apsum = ctx.enter_context(
            tc.tile_pool(name="apsum", bufs=1, space="PSUM"))

        # ---- constants ----
        # a2 here is "a2pad": 4 placement variants of the [c,8] score matrix,
        # variant w at cols [32w+8w_slot...]: A2 occupies cols 32w+8w..,
        # rest zeros (see _host_fold).  Slicing variant w gives a [128,32]
        # stationary whose zero columns make the scores land at the exact
        # psum-row slot the pooling matmul later needs -- so the transposed
        # p tile is directly usable as the pooling stationary.
        # The first transfers in the global DMA FIFO must be exactly what the
        # first score matmuls need, in consumption order: a2 (tiny), then xt
        # chunk 0 (both halves), then id16/ebias.  Hoisted here, ahead of the
        # xn prefetch configs on the gpsimd queue; the xt stash is consumed
        # by phase_a(t=0) of the first body.  Tail-only consts (ch 1MB,
        # id32, bout) are DMA'd from inside emit_body at x-stream end.
        a2_0 = consts.tile([128, 128], f8)
        a2_1 = consts.tile([128, 128], f8)
        nc.sync.dma_start(out=a2_0, in_=a2_d.ap()[0:128, :])
        nc.scalar.dma_start(out=a2_1, in_=a2_d.ap()[128:256, :])
        # first xt chunk halves ride the two HWDGE queues in parallel right
        # behind the (tiny) a2 transfers, so the first score matmuls can
        # start ~1us earlier than a serial gpsimd stash; gpsimd's first
        # transfer is then xn chunk 0, which pool(b=0) needs by ~3.4us.
        xt_stash = {}
        xt_stash[0] = (xt_pool.tile([128, XTCH * 128], f8, name="xt0"),
                       xt_pool.tile([128, XTCH * 128], f8, name="xt1"))
        nc.sync.dma_start(out=xt_stash[0][0],
                          in_=x_t.ap()[0:128, 0:XTCH * 128])
        nc.scalar.dma_start(out=xt_stash[0][1],
                            in_=x_t.ap()[128:256, 0:XTCH * 128])
        id16 = consts.tile([128, 128], f16)
        nc.sync.dma_start(out=id16, in_=id16_d.ap())
        ch_sb = consts.tile([128, 32 * 128], f16)
        bout_sb = consts.tile([128, 2], f32)
        tail_consts = [False]
        # exp bias: per-psum-row bias selecting live score slots (-U) vs dead
        # padding rows (-50 -> exp underflows to exactly 0 in fp16, so the
        # transposed p slices have true zeros outside each graph's slot).
        ebias = consts.tile([128, 2], f32)
        nc.scalar.dma_start(out=ebias, in_=ebias_d.ap())

        def emit_body():
            # ---- main streaming loop, pipelined in 4-tile half-groups ------
            # per half-group b (tiles 4b..4b+3 = 2 graphs):
            #   phase A: 8 score matmuls (a2pad stationary, M=32 at 32-aligned
            #            psum rows) -> sT128 [128,128] fully written, then ONE
            #            exp [128,128] -> p128 f16
            #   phase B (b-1): one [128,128] PE transpose; its output slices
            #            [:, 32j:32j+32] ARE the pooling stationaries (p at
            #            slot 8*(g%4), zeros elsewhere, from a2pad placement);
            #            4 Z-matmuls; drain every 8th half-group
            NB = NT // 4
            state = {}      # b -> (p128, xn_chunk)
            pending = []    # deferred drain transposes (PE work that must
                            # wait on the DVE/ACT drain chain; emitted one
                            # half-group later so PE never idles on it)
            xn_chunk = xt0 = xt1 = zp = None
            ap_list = []    # tail projection psum, partially accumulated
                            # mid-stream (windows 0..6) where PE has slack

            def proj_group(c0, c1):
                # CH-projection matmuls over znT window-columns [c0:c1);
                # one [128, 256] psum bank, a single accumulation group
                # spanning both output halves (two groups cannot share a
                # 2KB psum zero region)
                if not ap_list:
                    ap_list.append(apsum.tile([128, 256], f32, name="ap_ps"))
                for dh in range(2):
                    for h in range(H):
                        for c in range(2):
                            w = ch_sb[:, ((h * 2 + c) * 2 + dh) * 128:
                                      ((h * 2 + c) * 2 + dh) * 128 + 128]
                            rhs = znT[c].rearrange(
                                "p (k g h) -> p h (k g)", g=16, h=8)[
                                :, h, c0:c1]
                            nc.tensor.matmul(
                                ap_list[0][:, 128 * dh + c0:128 * dh + c1],
                                lhsT=w, rhs=rhs,
                                start=(dh == 0 and h == 0 and c == 0),
                                stop=(dh == 1 and h == H - 1 and c == 1))
            znT = [tailsb.tile([128, 8 * 128], f16, name=f"znT{c}",
                               tag=f"znT{c}") for c in range(2)]

            def dma_section(b):
                nonlocal xn_chunk, xt0, xt1
                t0 = 4 * b
                for t in range(t0, t0 + 4):
                    if t % XNCH == 0:
                        k = t // XNCH
                        xn_chunk = xn_pool.tile([128, XNCH, D + 1], f8,
                                                name="xn")
                        # Queue choice matters: the issuing sequencer is
                        # blocked ~600ns per dma_start configuring the DGE,
                        # which must not delay latency-critical compute issue
                        # (exp on Act releases PSUM for the next scores).
                        if "xn_dma" not in skip:
                            getattr(nc, qcfg[0]).dma_start(
                                out=xn_chunk.rearrange("p k c -> p (k c)"),
                                in_=x_nat.ap()[k])
                        else:
                            nc.gpsimd.memset(xn_chunk[:, 0, 0:1], 1.0)
                    if t % XTCH == 0:
                        k = t // XTCH
                        if k in xt_stash:
                            xt0, xt1 = xt_stash.pop(k)
                        else:
                            sl = slice(k * XTCH * 128, (k + 1) * XTCH * 128)
                            xt0 = xt_pool.tile([128, XTCH * 128], f8,
                                               name="xt0")
                            xt1 = xt_pool.tile([128, XTCH * 128], f8,
                                               name="xt1")
                            if "xt_dma" not in skip:
                                getattr(nc, qcfg[1]).dma_start(
                                    out=xt0, in_=x_t.ap()[0:128, sl])
                                getattr(nc, qcfg[2]).dma_start(
                                    out=xt1, in_=x_t.ap()[128:256, sl])
                            else:
                                nc.vector.memset(xt0[:, 0:1], 1.0)
                                nc.vector.memset(xt1[:, 0:1], 1.0)
                state[b] = [None, xn_chunk]

            def pool_mm(b, j):
                nonlocal zp
                if set(skip) & {"scores", "z"}:
                    return
                pT, xn_c = state[b]
                t = 4 * b + j
                if t % 32 == 0:
                    zp = zpsum.tile([128, D + 1], f32, name="zp")
                strip = (t % 32) // 8
                nc.tensor.matmul(zp[32 * strip:32 * strip + 32, :],
                                 lhsT=pT[:, 32 * j:32 * j + 32],
                                 rhs=xn_c[:, t % XNCH, :],
                                 start=(t % 8 == 0),
                                 stop=(t % 8 == 7),
                                 tile_position=(0, 32 * strip))

            def score_mm(b, j, half, sT):
                # a2pad stationary (M=32, exec-bound); scores land at the
                # psum-row slot the pooling matmul later needs, so the
                # transposed p tile is directly the pooling stationary
                if "scores" in skip:
                    return
                t = 4 * b + j
                w = (2 * b + j // 2) % 4
                lo = (t % XTCH) * 128
                a2c, xtc = (a2_0, xt0) if half == 0 else (a2_1, xt1)
                nc.tensor.matmul(
                    sT[32 * j:32 * j + 32, :],
                    lhsT=a2c[:, 32 * w:32 * w + 32],
                    rhs=xtc[:, lo:lo + 128],
                    start=(half == 0), stop=(half == 1),
                    tile_position=(0, 32 * j))

            def finish_scores(b, sT):
                if "scores" in skip:
                    return
                p128 = p_pool.tile([128, 128], f16, name="p128")
                nc.scalar.activation(out=p128, in_=sT, func=Exp,
                                     bias=ebias[:, b % 2:b % 2 + 1],
                                     scale=1.0 / 128.0)
                state[b][0] = p128

            def transpose_p(b):
                # PE transpose of p128(b) + DVE copy to SBUF, issued BEFORE
                # the next half-group's score matmuls so the copy's
                # PE->sem->DVE->sem->PE latency hides behind them.
                if "scores" in skip:
                    return
                p128, xn_c = state[b]
                tp = smallps.tile([128, 128], f16, name="tp", tag="tp")
                nc.tensor.transpose(tp, p128, id16)
                pT = p_pool.tile([128, 128], f16, name="pT", tag="pT")
                nc.vector.tensor_copy(pT, tp)
                state[b] = [pT, xn_c]

            def window_drain(b):
                # after the stop-matmul of a 32-tile psum window: normalize
                # and transpose Zn so only the CH projection remains later
                if set(skip) & {"scores", "z"}:
                    return
                if (4 * b) % 32 != 28:
                    return
                k = (4 * b) // 32
                rd = small.tile([128, 1], f32, name="rd")
                zn = zn_pool.tile([128, D], f16, name="zn")
                nc.vector.reciprocal(rd, zp[:, D:D + 1])
                nc.scalar.activation(out=zn, in_=zp[:, 0:D], func=Copy,
                                     bias=0.0, scale=rd)

                def drain(zn=zn, k=k):
                    for c in range(2):
                        tpz = smallps.tile([128, 128], f16, name="tpz",
                                           tag="tp")
                        nc.tensor.transpose(
                            tpz, zn[:, 128 * c:128 * (c + 1)], id16)
                        nc.vector.tensor_copy(
                            znT[c][:, 128 * k:128 * (k + 1)], tpz)
                pending.append(drain)

            # merged pipeline: scores(b) tile-interleaved with pooling(b-3).
            # Per-tile PE order s1 | pool[0:128] | s2 | pool[128:257] keeps
            # every >=128-cycle ldweights under a >=128-cycle execute.
            DEPTH_T, DEPTH_Z = 2, 3
            for b in range(NB + DEPTH_Z):
                if b == NB - 8 and not tail_consts[0]:
                    # tail-only consts ~7us before stream end, on the gpsimd
                    # queue between xn chunks (vector carries odd chunks)
                    tail_consts[0] = True
                    nc.gpsimd.dma_start(out=ch_sb, in_=ch_d.ap())
                    nc.gpsimd.dma_start(out=bout_sb, in_=bout_d.ap())
                # deferred window-drain transposes: emitted one half-group
                # after the recip->copy chain so PE never stalls on it
                while pending:
                    pending.pop(0)()
                sT = None
                if b < NB:
                    dma_section(b)
                    if "scores" not in skip:
                        sT = bigps.tile([128, 128], f32, name="sT",
                                        tag="bigps")
                # keep same-config matmuls contiguous: fine-grained
                # interleaving of different-shape matmuls measures ~2.5x
                # slower on hardware
                zb = b - DEPTH_Z
                if DEPTH_T <= b < NB + DEPTH_T:
                    transpose_p(b - DEPTH_T)
                for j in range(4):
                    if b < NB:
                        score_mm(b, j, 0, sT)
                        score_mm(b, j, 1, sT)
                for j in range(4):
                    if zb >= 0:
                        pool_mm(zb, j)
                if b < NB:
                    finish_scores(b, sT)
                if zb >= 0:
                    window_drain(zb)
                    state.pop(zb, None)

            if set(skip) & {"scores", "z"}:
                # diagnostic variant: no drains/tail; emit a token output DMA
                nc.sync.dma_start(out=out_d.ap()[0:1, 0:2],
                                  in_=ebias[0:1, 0:2])
                return

            # ---- tail: project with folded CH, add bias ----
            while pending:
                pending.pop(0)()
            # the act output [e, g] ships directly as transposed output rows
            proj_group(0, 128)
            for dh in range(2):
                atb = tailsb.tile([128, 128], f32, name=f"atb{dh}")
                nc.scalar.activation(out=atb,
                                     in_=ap_list[0][:, 128 * dh:
                                                    128 * dh + 128],
                                     func=Ident,
                                     bias=bout_sb[:, dh:dh + 1], scale=1.0)
                if tiny_out:
                    if dh == 0:
                        nc.sync.dma_start(out=out_d.ap(), in_=atb[0:1, 0:4])
                else:
                    nc.sync.dma_start(
                        out=out_d.ap()[128 * dh:128 * (dh + 1), :], in_=atb)

        if loop_iters:
            with tc.For_i(0, loop_iters, 1):
                emit_body()
        else:
            for _rep in range(repeats):
                emit_body()

    nc.compile()
    return nc


def _host_fold(query, in_proj_weight, in_proj_bias, out_proj_weight,
               out_proj_bias, xt_f8=True):
    W = np.asarray(in_proj_weight, np.float64)
    Wq, Wk, Wv = W[:D], W[D:2 * D], W[2 * D:]
    b = np.asarray(in_proj_bias, np.float64)
    bq = b[:D]
    Wout = np.asarray(out_proj_weight, np.float64)
    scale = 1.0 / np.sqrt(DH)
    q = (np.asarray(query, np.float64).reshape(D) @ Wq.T + bq).reshape(H, DH)
    A2 = np.zeros((D, H))
    for h in range(H):
        A2[:, h] = scale * (Wk[h * DH:(h + 1) * DH, :].T @ q[h])
    U = 4.5 * float(np.linalg.norm(A2, axis=0).max())
    # a2pad: 4 placement variants; variant w (cols 32w..32w+32) has A2 at
    # within-block cols [8w, 8w+8), zeros elsewhere.  Stored as e3m4 x64 so
    # the ~0.02-magnitude entries sit in the normal range (min normal 0.25);
    # the device undoes the combined x128 (with the x2 on x_t) via the exp
    # activation's scale.
    import ml_dtypes
    if xt_f8:
        A2q = (A2 * 64.0).astype(ml_dtypes.float8_e3m4).astype(np.float64)
    else:
        A2q = A2
    a2p = np.zeros((D, 128))
    for w in range(4):
        a2p[:, 32 * w + 8 * w:32 * w + 8 * w + H] = A2q
    # exp bias patterns: live slots get -U, dead rows -50 (exp -> fp16 zero).
    # half-group b even: tiles j<2 -> slot 0, j>=2 -> slot 1;
    # b odd: slots 2 and 3.  Row = 32j + 8*slot + h.
    ebias = np.full((128, 2), -50.0)
    for col, (w01, w23) in enumerate(((0, 1), (2, 3))):
        for j in range(4):
            w = w01 if j < 2 else w23
            ebias[32 * j + 8 * w:32 * j + 8 * w + H, col] = -U
    # ch layout: [c_in_half, (h, c_half, d_half, d_in_half)]
    ch = np.zeros((128, 32 * 128), np.float64)
    for h in range(H):
        CH_h = Wv[h * DH:(h + 1) * DH, :].T @ Wout[:, h * DH:(h + 1) * DH].T
        for c in range(2):
            for dh in range(2):
                blk = CH_h[c * 128:(c + 1) * 128, dh * 128:(dh + 1) * 128]
                ch[:, ((h * 2 + c) * 2 + dh) * 128:
                   ((h * 2 + c) * 2 + dh) * 128 + 128] = blk
    bout = np.asarray(out_proj_bias, np.float64)
    bout2 = np.stack([bout[:128], bout[128:]], axis=1)  # [128, 2]
    return a2p, ebias, U, ch, bout2


def _make_in_maps(x, a2p, ebias, ch, bout2, xnch=8, xt_f8=True):
    import ml_dtypes
    f8e3 = ml_dtypes.float8_e3m4
    a2_8 = a2p.astype(f8e3)
    ebias_32 = ebias.astype(np.float32)
    ch_16 = ch.astype(np.float16)
    bout_32 = bout2.astype(np.float32)
    id16 = np.eye(128, dtype=np.float16)
    in_maps = []
    for c in range(N_CORES):
        xs = x[c * N_LOC:(c + 1) * N_LOC]
        # pooling stream in fp8 e3m4: x is scaled x2 to sit in the normal
        # range; the ones column is 2.0 so the denominator carries the same
        # factor and it cancels in the drain's Z/denom.
        x_nat = np.empty((N_LOC, D + 1), f8e3)
        x_nat[:, :D] = (xs.astype(np.float64) * 2.0).astype(f8e3)
        x_nat[:, D] = 2.0
        # pre-tile: row (128t+p) -> [chunk t//xnch, partition p, t%xnch, c]
        x_nat = np.ascontiguousarray(
            x_nat.reshape(NT // xnch, xnch, 128, D + 1).transpose(0, 2, 1, 3)
        ).reshape(NT // xnch, 128, xnch * (D + 1))
        x_tp = np.ascontiguousarray(
            (xs.T.astype(np.float64) * 2.0)).astype(f8e3)
        in_maps.append({
            "x_nat": x_nat, "x_t": x_tp, "a2": a2_8, "ebias": ebias_32,
            "ch": ch_16, "bout": bout_32, "id16": id16,
        })
    return in_maps


def kernel(x, batch, query, in_proj_weight, in_proj_bias, out_proj_weight,
           out_proj_bias, num_heads, num_graphs):
    from concourse import bass_utils

    x = np.asarray(x, np.float32)
    batch = np.asarray(batch)
    assert x.shape == (N_TOTAL, D) and int(num_heads) == H
    assert int(num_graphs) == G_TOTAL
    expected = (np.arange(N_TOTAL, dtype=np.int64) * G_TOTAL) // N_TOTAL
    assert np.array_equal(batch.astype(np.int64), expected), \
        "kernel compiled for equal-sized segments"

    a2p, ebias, U, ch, bout2 = _host_fold(query, in_proj_weight,
                                          in_proj_bias, out_proj_weight,
                                          out_proj_bias)

    key = round(U, 9)
    if key not in _CACHE:
        _CACHE[key] = _build_program(U)
    nc = _CACHE[key]

    in_maps = _make_in_maps(x, a2p, ebias, ch, bout2)
    global _last_in_maps
    _last_in_maps = in_maps
    res = bass_utils.run_bass_kernel_spmd(nc, in_maps,
                                          core_ids=list(range(N_CORES)))
    # device writes [D, G_LOC] per core; un-transpose on the host
    out = np.concatenate([r["out"].T for r in res.results], axis=0)
    return np.ascontiguousarray(out, dtype=np.float32)



# revision 71
# speedup vs baseline: 1.0300x; 1.0300x over previous
"""Trainium2 Bass kernel for single-query cross-attention pooling
(segment softmax over equal-sized node segments, 8-way node/graph sharded).

Self-contained: hardcodes N=262144, D=256, H=8, G=1024, 8 cores.

Algorithm (per core, 128 graphs / 32768 nodes, all device math in fp16 with
fp32 PSUM accumulation):
  scores[n,h] = x[n,:] @ A2[:,h]        A2 = scale * Wk_h^T q_h  (host-folded;
                                        per-head additive consts cancel in the
                                        segment softmax, so they are dropped)
  p = exp(scores - U)                   U = uniform host bound, cancels too
  Z[g,h,:] = sum_{n in g} p[n,h]*[x[n,:] | 1]   (pool BEFORE projecting:
                                        projection commutes with the pooled sum)
  out[g,:] = sum_h (Z[g,h,:256]/Z[g,h,256]) @ CH_h + bout
                                        CH_h = Wv_h^T Wout_h^T  (host-folded)

The transposed copy of x (xT) is prepared on the host so scores can be computed
with x-tiles as the matmul stationary operand -> node-major scores, and no
on-device transposes of x are ever needed.  Total HBM traffic per core is
2 * 16 MB fp16 = the fp32-read-once roofline.
"""

import numpy as np

N_TOTAL = 262144
D = 256
H = 8
DH = 32
G_TOTAL = 1024
N_CORES = 8

N_LOC = N_TOTAL // N_CORES          # 32768 nodes per core
G_LOC = G_TOTAL // N_CORES          # 128 graphs per core
SEG = N_TOTAL // G_TOTAL            # 256 nodes per graph (equal segments)
NT = N_LOC // 128                   # 256 node tiles per core

_CACHE = {}


def _build_program(U, repeats=1, loop_iters=0, skip=(), tiny_out=False,
                   xnch=8, xtch=16, xt_f8=True, qcfg=("gpsimd", "sync",
                                                      "scalar")):
    """skip: subset of {'scores','z','xt_dma','xn_dma'} - diagnostic variants
    that elide parts of the per-tile work (output garbage; timing only).
    tiny_out: shrink the output tensor to [1,4] so relay transport per call
    is negligible (timing variants)."""
    import concourse.bass as bass
    import concourse.tile as tile
    from concourse import bacc, mybir
    from contextlib import ExitStack

    f16 = mybir.dt.float16
    f32 = mybir.dt.float32
    f8 = mybir.dt.float8e3 if xt_f8 else mybir.dt.float16
    f8e4 = mybir.dt.float8e4
    DR = mybir.MatmulPerfMode.DoubleRow
    import os
    xn_bufs = int(os.environ.get("XN_BUFS", "8"))
    xt_bufs = int(os.environ.get("XT_BUFS", "6"))

    nc = bacc.Bacc("TRN2", target_bir_lowering=False, debug=False,
                   num_devices=N_CORES)

    # x_nat is host-pre-tiled: [chunk, partition, tile_in_chunk * (D+1)] so a
    # chunk DMA is 128 contiguous 8224-byte descriptors instead of 2048
    # small ones (HWDGE descriptor-generation was the SP-seq bottleneck).
    XNCH = xnch          # x_nat tiles per DMA chunk
    XTCH = xtch          # xT tiles per DMA chunk
    x_nat = nc.dram_tensor("x_nat", [NT // XNCH, 128, XNCH * (D + 1)], f8,
                           kind="ExternalInput")
    # x_t / a2 are fp8-e3m4, pre-scaled x2 / x64 on the host; the combined
    # x128 on the scores is undone by the exp activation's scale=1/128.
    x_t = nc.dram_tensor("x_t", [D, N_LOC], f8, kind="ExternalInput")
    a2_d = nc.dram_tensor("a2", [D, 128], f8, kind="ExternalInput")
    ch_d = nc.dram_tensor("ch", [128, 32 * 128], f16, kind="ExternalInput")
    bout_d = nc.dram_tensor("bout", [128, 2], f32, kind="ExternalInput")
    ebias_d = nc.dram_tensor("ebias", [128, 2], f32, kind="ExternalInput")
    id16_d = nc.dram_tensor("id16", [128, 128], f16, kind="ExternalInput")
    # output is written TRANSPOSED [D, G_LOC] straight from the projection
    # psum (saves the final PE transposes); the host un-transposes.
    out_shape = [1, 4] if tiny_out else [D, G_LOC]
    out_d = nc.dram_tensor("out", out_shape, f32, kind="ExternalOutput")

    Exp = mybir.ActivationFunctionType.Exp
    Copy = mybir.ActivationFunctionType.Copy
    Ident = mybir.ActivationFunctionType.Identity

    with tile.TileContext(nc) as tc, ExitStack() as ctx:
        consts = ctx.enter_context(tc.tile_pool(name="consts", bufs=1))
        xn_pool = ctx.enter_context(tc.tile_pool(name="xn_pool", bufs=xn_bufs))
        xt_pool = ctx.enter_context(tc.tile_pool(name="xt_pool", bufs=xt_bufs))
        zn_pool = ctx.enter_context(tc.tile_pool(name="zn_pool", bufs=2))
        p_pool = ctx.enter_context(tc.tile_pool(name="p_pool", bufs=8))
        small = ctx.enter_context(tc.tile_pool(name="small", bufs=8))
        tailsb = ctx.enter_context(tc.tile_pool(name="tailsb", bufs=2))
        # PSUM: 8 banks total; tags share slots across main loop and tail
        bigps = ctx.enter_context(
            tc.tile_pool(name="bigps", bufs=2, space="PSUM"))
        smallps = ctx.enter_context(
            tc.tile_pool(name="smallps", bufs=2, space="PSUM"))
        zpsum = ctx.enter_context(
            tc.tile_pool(name="zpsum", bufs=2, space="PSUM"))
        apsum = ctx.enter_context(
            tc.tile_pool(name="apsum", bufs=1, space="PSUM"))

        # ---- constants ----
        # a2 here is "a2pad": 4 placement variants of the [c,8] score matrix,
        # variant w at cols [32w+8w_slot...]: A2 occupies cols 32w+8w..,
        # rest zeros (see _host_fold).  Slicing variant w gives a [128,32]
        # stationary whose zero columns make the scores land at the exact
        # psum-row slot the pooling matmul later needs -- so the transposed
        # p tile is directly usable as the pooling stationary.
        # The first transfers in the global DMA FIFO must be exactly what the
        # first score matmuls need, in consumption order: a2 (tiny), then xt
        # chunk 0 (both halves), then id16/ebias.  Hoisted here, ahead of the
        # xn prefetch configs on the gpsimd queue; the xt stash is consumed
        # by phase_a(t=0) of the first body.  Tail-only consts (ch 1MB,
        # id32, bout) are DMA'd from inside emit_body at x-stream end.
        a2_0 = consts.tile([128, 128], f8)
        a2_1 = consts.tile([128, 128], f8)
        nc.sync.dma_start(out=a2_0, in_=a2_d.ap()[0:128, :])
        nc.scalar.dma_start(out=a2_1, in_=a2_d.ap()[128:256, :])
        # first xt chunk halves ride the two HWDGE queues in parallel right
        # behind the (tiny) a2 transfers, so the first score matmuls can
        # start ~1us earlier than a serial gpsimd stash; gpsimd's first
        # transfer is then xn chunk 0, which pool(b=0) needs by ~3.4us.
        xt_stash = {}
        xt_stash[0] = (xt_pool.tile([128, XTCH * 128], f8, name="xt0"),
                       xt_pool.tile([128, XTCH * 128], f8, name="xt1"))
        nc.sync.dma_start(out=xt_stash[0][0],
                          in_=x_t.ap()[0:128, 0:XTCH * 128])
        nc.scalar.dma_start(out=xt_stash[0][1],
                            in_=x_t.ap()[128:256, 0:XTCH * 128])
        id16 = consts.tile([128, 128], f16)
        nc.sync.dma_start(out=id16, in_=id16_d.ap())
        ch_sb = consts.tile([128, 32 * 128], f16)
        bout_sb = consts.tile([128, 2], f32)
        tail_consts = [False]
        # exp bias: per-psum-row bias selecting live score slots (-U) vs dead
        # padding rows (-50 -> exp underflows to exactly 0 in fp16, so the
        # transposed p slices have true zeros outside each graph's slot).
        ebias = consts.tile([128, 2], f32)
        nc.scalar.dma_start(out=ebias, in_=ebias_d.ap())

        def emit_body():
            # ---- main streaming loop, pipelined in 4-tile half-groups ------
            # per half-group b (tiles 4b..4b+3 = 2 graphs):
            #   phase A: 8 score matmuls (a2pad stationary, M=32 at 32-aligned
            #            psum rows) -> sT128 [128,128] fully written, then ONE
            #            exp [128,128] -> p128 f16
            #   phase B (b-1): one [128,128] PE transpose; its output slices
            #            [:, 32j:32j+32] ARE the pooling stationaries (p at
            #            slot 8*(g%4), zeros elsewhere, from a2pad placement);
            #            4 Z-matmuls; drain every 8th half-group
            NB = NT // 4
            state = {}      # b -> (p128, xn_chunk)
            pending = []    # deferred drain transposes (PE work that must
                            # wait on the DVE/ACT drain chain; emitted one
                            # half-group later so PE never idles on it)
            xn_chunk = xt0 = xt1 = zp = None
            ap_list = []    # tail projection psum, partially accumulated
                            # mid-stream (windows 0..6) where PE has slack

            def proj_group(c0, c1):
                # CH-projection matmuls over znT window-columns [c0:c1);
                # one [128, 256] psum bank, a single accumulation group
                # spanning both output halves (two groups cannot share a
                # 2KB psum zero region)
                if not ap_list:
                    ap_list.append(apsum.tile([128, 256], f32, name="ap_ps"))
                for dh in range(2):
                    for h in range(H):
                        for c in range(2):
                            w = ch_sb[:, ((h * 2 + c) * 2 + dh) * 128:
                                      ((h * 2 + c) * 2 + dh) * 128 + 128]
                            rhs = znT[c].rearrange(
                                "p (k g h) -> p h (k g)", g=16, h=8)[
                                :, h, c0:c1]
                            nc.tensor.matmul(
                                ap_list[0][:, 128 * dh + c0:128 * dh + c1],
                                lhsT=w, rhs=rhs,
                                start=(dh == 0 and h == 0 and c == 0),
                                stop=(dh == 1 and h == H - 1 and c == 1))
            znT = [tailsb.tile([128, 8 * 128], f16, name=f"znT{c}",
                               tag=f"znT{c}") for c in range(2)]

            def dma_section(b):
                nonlocal xn_chunk, xt0, xt1
                t0 = 4 * b
                for t in range(t0, t0 + 4):
                    if t % XNCH == 0:
                        k = t // XNCH
                        xn_chunk = xn_pool.tile([128, XNCH, D + 1], f8,
                                                name="xn")
                        # Queue choice matters: the issuing sequencer is
                        # blocked ~600ns per dma_start configuring the DGE,
                        # which must not delay latency-critical compute issue
                        # (exp on Act releases PSUM for the next scores).
                        if "xn_dma" not in skip:
                            getattr(nc, qcfg[0]).dma_start(
                                out=xn_chunk.rearrange("p k c -> p (k c)"),
                                in_=x_nat.ap()[k])
                        else:
                            nc.gpsimd.memset(xn_chunk[:, 0, 0:1], 1.0)
                    if t % XTCH == 0:
                        k = t // XTCH
                        if k in xt_stash:
                            xt0, xt1 = xt_stash.pop(k)
                        else:
                            sl = slice(k * XTCH * 128, (k + 1) * XTCH * 128)
                            xt0 = xt_pool.tile([128, XTCH * 128], f8,
                                               name="xt0")
                            xt1 = xt_pool.tile([128, XTCH * 128], f8,
                                               name="xt1")
                            if "xt_dma" not in skip:
                                getattr(nc, qcfg[1]).dma_start(
                                    out=xt0, in_=x_t.ap()[0:128, sl])
                                getattr(nc, qcfg[2]).dma_start(
                                    out=xt1, in_=x_t.ap()[128:256, sl])
                            else:
                                nc.vector.memset(xt0[:, 0:1], 1.0)
                                nc.vector.memset(xt1[:, 0:1], 1.0)
                state[b] = [None, xn_chunk]

            def pool_mm(b, j):
                nonlocal zp
                if set(skip) & {"scores", "z"}:
                    return
                pT, xn_c = state[b]
                t = 4 * b + j
                if t % 32 == 0:
                    zp = zpsum.tile([128, D + 1], f32, name="zp")
                strip = (t % 32) // 8
                nc.tensor.matmul(zp[32 * strip:32 * strip + 32, :],
                                 lhsT=pT[:, 32 * j:32 * j + 32],
                                 rhs=xn_c[:, t % XNCH, :],
                                 start=(t % 8 == 0),
                                 stop=(t % 8 == 7),
                                 tile_position=(0, 32 * strip))

            def score_mm(b, j, half, sT):
                # a2pad stationary (M=32, exec-bound); scores land at the
                # psum-row slot the pooling matmul later needs, so the
                # transposed p tile is directly the pooling stationary
                if "scores" in skip:
                    return
                t = 4 * b + j
                w = (2 * b + j // 2) % 4
                lo = (t % XTCH) * 128
                a2c, xtc = (a2_0, xt0) if half == 0 else (a2_1, xt1)
                nc.tensor.matmul(
                    sT[32 * j:32 * j + 32, :],
                    lhsT=a2c[:, 32 * w:32 * w + 32],
                    rhs=xtc[:, lo:lo + 128],
                    start=(half == 0), stop=(half == 1),
                    tile_position=(0, 32 * j))

            def finish_scores(b, sT):
                if "scores" in skip:
                    return
                p128 = p_pool.tile([128, 128], f16, name="p128")
                nc.scalar.activation(out=p128, in_=sT, func=Exp,
                                     bias=ebias[:, b % 2:b % 2 + 1],
                                     scale=1.0 / 128.0)
                state[b][0] = p128

            def transpose_p(b):
                # PE transpose of p128(b) + DVE copy to SBUF, issued BEFORE
                # the next half-group's score matmuls so the copy's
                # PE->sem->DVE->sem->PE latency hides behind them.
                if "scores" in skip:
                    return
                p128, xn_c = state[b]
                tp = smallps.tile([128, 128], f16, name="tp", tag="tp")
                nc.tensor.transpose(tp, p128, id16)
                pT = p_pool.tile([128, 128], f16, name="pT", tag="pT")
                nc.vector.tensor_copy(pT, tp)
                state[b] = [pT, xn_c]

            def window_drain(b):
                # after the stop-matmul of a 32-tile psum window: normalize
                # and transpose Zn so only the CH projection remains later
                if set(skip) & {"scores", "z"}:
                    return
                if (4 * b) % 32 != 28:
                    return
                k = (4 * b) // 32
                rd = small.tile([128, 1], f32, name="rd")
                zn = zn_pool.tile([128, D], f16, name="zn")
                nc.vector.reciprocal(rd, zp[:, D:D + 1])
                nc.scalar.activation(out=zn, in_=zp[:, 0:D], func=Copy,
                                     bias=0.0, scale=rd)

                def drain(zn=zn, k=k):
                    for c in range(2):
                        tpz = smallps.tile([128, 128], f16, name="tpz",
                                           tag="tp")
                        nc.tensor.transpose(
                            tpz, zn[:, 128 * c:128 * (c + 1)], id16)
                        nc.vector.tensor_copy(
                            znT[c][:, 128 * k:128 * (k + 1)], tpz)
                pending.append(drain)

            # merged pipeline: scores(b) tile-interleaved with pooling(b-3).
            # Per-tile PE order s1 | pool[0:128] | s2 | pool[128:257] keeps
            # every >=128-cycle ldweights under a >=128-cycle execute.
            DEPTH_T, DEPTH_Z = 2, 4
            for b in range(NB + DEPTH_Z):
                if b == NB - 8 and not tail_consts[0]:
                    # tail-only consts ~7us before stream end, on the gpsimd
                    # queue between xn chunks (vector carries odd chunks)
                    tail_consts[0] = True
                    nc.gpsimd.dma_start(out=ch_sb, in_=ch_d.ap())
                    nc.gpsimd.dma_start(out=bout_sb, in_=bout_d.ap())
                # deferred window-drain transposes: emitted one half-group
                # after the recip->copy chain so PE never stalls on it
                while pending:
                    pending.pop(0)()
                sT = None
                if b < NB:
                    dma_section(b)
                    if "scores" not in skip:
                        sT = bigps.tile([128, 128], f32, name="sT",
                                        tag="bigps")
                # keep same-config matmuls contiguous: fine-grained
                # interleaving of different-shape matmuls measures ~2.5x
                # slower on hardware
                zb = b - DEPTH_Z
                # batch p-transposes in pairs on even iterations: PE
                # tile-config switches ((128,128) transpose vs (128,32)
                # score/pool matmuls) are expensive on HW, so halve their
                # count by grouping same-config work
                if b % 2 == 0:
                    for tb in (b - DEPTH_T - 1, b - DEPTH_T):
                        if 0 <= tb < NB:
                            transpose_p(tb)
                for j in range(4):
                    if b < NB:
                        score_mm(b, j, 0, sT)
                        score_mm(b, j, 1, sT)
                for j in range(4):
                    if zb >= 0:
                        pool_mm(zb, j)
                if b < NB:
                    finish_scores(b, sT)
                if zb >= 0:
                    window_drain(zb)
                    state.pop(zb, None)

            if set(skip) & {"scores", "z"}:
                # diagnostic variant: no drains/tail; emit a token output DMA
                nc.sync.dma_start(out=out_d.ap()[0:1, 0:2],
                                  in_=ebias[0:1, 0:2])
                return

            # ---- tail: project with folded CH, add bias ----
            while pending:
                pending.pop(0)()
            # the act output [e, g] ships directly as transposed output rows
            proj_group(0, 128)
            for dh in range(2):
                atb = tailsb.tile([128, 128], f32, name=f"atb{dh}")
                nc.scalar.activation(out=atb,
                                     in_=ap_list[0][:, 128 * dh:
                                                    128 * dh + 128],
                                     func=Ident,
                                     bias=bout_sb[:, dh:dh + 1], scale=1.0)
                if tiny_out:
                    if dh == 0:
                        nc.sync.dma_start(out=out_d.ap(), in_=atb[0:1, 0:4])
                else:
                    nc.sync.dma_start(
                        out=out_d.ap()[128 * dh:128 * (dh + 1), :], in_=atb)

        if loop_iters:
            with tc.For_i(0, loop_iters, 1):
                emit_body()
        else:
            for _rep in range(repeats):
                emit_body()

    nc.compile()
    return nc


def _host_fold(query, in_proj_weight, in_proj_bias, out_proj_weight,
               out_proj_bias, xt_f8=True):
    W = np.asarray(in_proj_weight, np.float64)
    Wq, Wk, Wv = W[:D], W[D:2 * D], W[2 * D:]
    b = np.asarray(in_proj_bias, np.float64)
    bq = b[:D]
    Wout = np.asarray(out_proj_weight, np.float64)
    scale = 1.0 / np.sqrt(DH)
    q = (np.asarray(query, np.float64).reshape(D) @ Wq.T + bq).reshape(H, DH)
    A2 = np.zeros((D, H))
    for h in range(H):
        A2[:, h] = scale * (Wk[h * DH:(h + 1) * DH, :].T @ q[h])
    U = 4.5 * float(np.linalg.norm(A2, axis=0).max())
    # a2pad: 4 placement variants; variant w (cols 32w..32w+32) has A2 at
    # within-block cols [8w, 8w+8), zeros elsewhere.  Stored as e3m4 x64 so
    # the ~0.02-magnitude entries sit in the normal range (min normal 0.25);
    # the device undoes the combined x128 (with the x2 on x_t) via the exp
    # activation's scale.
    import ml_dtypes
    if xt_f8:
        A2q = (A2 * 64.0).astype(ml_dtypes.float8_e3m4).astype(np.float64)
    else:
        A2q = A2
    a2p = np.zeros((D, 128))
    for w in range(4):
        a2p[:, 32 * w + 8 * w:32 * w + 8 * w + H] = A2q
    # exp bias patterns: live slots get -U, dead rows -50 (exp -> fp16 zero).
    # half-group b even: tiles j<2 -> slot 0, j>=2 -> slot 1;
    # b odd: slots 2 and 3.  Row = 32j + 8*slot + h.
    ebias = np.full((128, 2), -50.0)
    for col, (w01, w23) in enumerate(((0, 1), (2, 3))):
        for j in range(4):
            w = w01 if j < 2 else w23
            ebias[32 * j + 8 * w:32 * j + 8 * w + H, col] = -U
    # ch layout: [c_in_half, (h, c_half, d_half, d_in_half)]
    ch = np.zeros((128, 32 * 128), np.float64)
    for h in range(H):
        CH_h = Wv[h * DH:(h + 1) * DH, :].T @ Wout[:, h * DH:(h + 1) * DH].T
        for c in range(2):
            for dh in range(2):
                blk = CH_h[c * 128:(c + 1) * 128, dh * 128:(dh + 1) * 128]
                ch[:, ((h * 2 + c) * 2 + dh) * 128:
                   ((h * 2 + c) * 2 + dh) * 128 + 128] = blk
    bout = np.asarray(out_proj_bias, np.float64)
    bout2 = np.stack([bout[:128], bout[128:]], axis=1)  # [128, 2]
    return a2p, ebias, U, ch, bout2


def _make_in_maps(x, a2p, ebias, ch, bout2, xnch=8, xt_f8=True):
    import ml_dtypes
    f8e3 = ml_dtypes.float8_e3m4
    a2_8 = a2p.astype(f8e3)
    ebias_32 = ebias.astype(np.float32)
    ch_16 = ch.astype(np.float16)
    bout_32 = bout2.astype(np.float32)
    id16 = np.eye(128, dtype=np.float16)
    in_maps = []
    for c in range(N_CORES):
        xs = x[c * N_LOC:(c + 1) * N_LOC]
        # pooling stream in fp8 e3m4: x is scaled x2 to sit in the normal
        # range; the ones column is 2.0 so the denominator carries the same
        # factor and it cancels in the drain's Z/denom.
        x_nat = np.empty((N_LOC, D + 1), f8e3)
        x_nat[:, :D] = (xs.astype(np.float64) * 2.0).astype(f8e3)
        x_nat[:, D] = 2.0
        # pre-tile: row (128t+p) -> [chunk t//xnch, partition p, t%xnch, c]
        x_nat = np.ascontiguousarray(
            x_nat.reshape(NT // xnch, xnch, 128, D + 1).transpose(0, 2, 1, 3)
        ).reshape(NT // xnch, 128, xnch * (D + 1))
        x_tp = np.ascontiguousarray(
            (xs.T.astype(np.float64) * 2.0)).astype(f8e3)
        in_maps.append({
            "x_nat": x_nat, "x_t": x_tp, "a2": a2_8, "ebias": ebias_32,
            "ch": ch_16, "bout": bout_32, "id16": id16,
        })
    return in_maps


def kernel(x, batch, query, in_proj_weight, in_proj_bias, out_proj_weight,
           out_proj_bias, num_heads, num_graphs):
    from concourse import bass_utils

    x = np.asarray(x, np.float32)
    batch = np.asarray(batch)
    assert x.shape == (N_TOTAL, D) and int(num_heads) == H
    assert int(num_graphs) == G_TOTAL
    expected = (np.arange(N_TOTAL, dtype=np.int64) * G_TOTAL) // N_TOTAL
    assert np.array_equal(batch.astype(np.int64), expected), \
        "kernel compiled for equal-sized segments"

    a2p, ebias, U, ch, bout2 = _host_fold(query, in_proj_weight,
                                          in_proj_bias, out_proj_weight,
                                          out_proj_bias)

    key = round(U, 9)
    if key not in _CACHE:
        _CACHE[key] = _build_program(U)
    nc = _CACHE[key]

    in_maps = _make_in_maps(x, a2p, ebias, ch, bout2)
    global _last_in_maps
    _last_in_maps = in_maps
    res = bass_utils.run_bass_kernel_spmd(nc, in_maps,
                                          core_ids=list(range(N_CORES)))
    # device writes [D, G_LOC] per core; un-transpose on the host
    out = np.concatenate([r["out"].T for r in res.results], axis=0)
    return np.ascontiguousarray(out, dtype=np.float32)

